# revision 2
# baseline (speedup 1.0000x reference)
"""CosineAttention Trainium2 kernel (8 NeuronCores, SPMD).

Sharding: 16 (batch, head) pairs -> 8 cores, 2 heads (one batch) per core.
Per core, attention runs in transposed-score layout (scoresT[kpos, qpos]) so
both attention matmuls contract over the partition dim with no transposes:
  MM1: scoresT = kT.T-slice.T @ qT-slice        (K=32 head_dim)
  exp: ACT Exp with per-partition scale=(k pixel-norm * 1/sqrt(hd)) and
       bias=ln(v pixel-norm) folded in -> e = sv[k]*exp(true scoreT)
  MM2: lhsT = [v_hat | 1/sv] (M=33): accumulates y^T rows and the softmax
       denominator row in one PSUM accumulation group.
Weight-norm scales fold into conv PSUM evictions (per-partition scalars).
The out-conv is computed per-core on the core's 64 attention channels; the
8 partial results (each including 1/4 of the x residual term) are summed on
host per batch -- that sum is the gather/unshard step.
"""
import sys
import types

import numpy as np

try:  # this image's antenv may lack axon_hooks; stub it so tracing degrades
    import antenv.axon_hooks  # noqa: F401
except ImportError:
    import antenv
    _m = types.ModuleType("antenv.axon_hooks")
    _m.get_axon_ntff_profile_hook = lambda: None
    _m.set_axon_ntff_profile_hook = lambda h: None
    sys.modules["antenv.axon_hooks"] = _m
    antenv.axon_hooks = _m
except Exception:
    pass

import concourse.bass as bass
import concourse.tile as tile
from concourse import mybir
from concourse.bass_utils import run_bass_kernel_spmd

F32 = mybir.dt.float32
AF = mybir.ActivationFunctionType
ALU = mybir.AluOpType

EPS = 1e-4
MP_T = 0.3
INV_SCALE = 1.0 / np.sqrt(MP_T ** 2 + (1.0 - MP_T) ** 2)
C = 256          # channels
HW = 4096        # pixels
HD = 32          # head dim
NCH = 8          # 512-wide pixel chunks
KT = 32          # 128-wide kpos tiles
LOG_ISQ_HD = float(np.log(1.0 / np.sqrt(HD)))
C_X = 0.25 * (1.0 - MP_T) * INV_SCALE     # per-core share of residual
C_Y = MP_T * INV_SCALE                    # folded into w_out scale
W_EPS = 16.0 * EPS                        # sqrt(fan_in)*EPS with fan_in=256


def _split_waits(nc):
    """This walrus accepts 1 sync wait per engine instruction: hoist extras
    into preceding NoOps on the same engine (engines are in-order)."""
    for f in nc.m.functions:
        for bb in f.blocks:
            newlist = []
            for inst in bb.instructions:
                si = inst.sync_info
                if si is not None and si.on_wait is not None and len(si.on_wait) > 1:
                    waits = list(si.on_wait)
                    if "DMA" in type(inst).__name__:
                        # keep the compute-engine sem on the DMA descriptor;
                        # hoist DMA-queue sems (monotonic, engine-stall safe)
                        hw = [w for w in waits if str(w.ant_name).startswith("DMA")]
                        eng = [w for w in waits if not str(w.ant_name).startswith("DMA")]
                        if eng:
                            keep, extra = eng[-1:], hw + eng[:-1]
                        else:
                            keep, extra = hw[-1:], hw[:-1]
                    else:
                        extra, keep = waits[:-1], waits[-1:]
                    for idx, w in enumerate(extra):
                        nop = mybir.InstNoOp(
                            name=f"{inst.name}_ws{idx}", ins=[], outs=[],
                            sync_info=mybir.SyncInfo(on_wait=[w], on_update=[]))
                        nop.engine = inst.engine
                        newlist.append(nop)
                    inst.sync_info = mybir.SyncInfo(
                        on_wait=keep, on_update=list(si.on_update or []))
                newlist.append(inst)
            bb.instructions = newlist


def _weight_scale_rows(nc, work, nat_ap, p):
    """Per-row weight-norm scale s = 1/(||w_row|| + 16*eps) for natural-layout
    [p, 256] weight rows. Returns a [p, 1] sbuf AP."""
    sq = work.tile([p, 256], F32, tag="wsq", name=f"wsq_{nc.next_id()}")
    nc.vector.tensor_mul(sq, nat_ap, nat_ap)
    ssq = work.tile([p, 1], F32, tag="wssq", name=f"wssq_{nc.next_id()}")
    nc.vector.tensor_reduce(ssq, sq, axis=mybir.AxisListType.X, op=ALU.add)
    ln = work.tile([p, 1], F32, tag="wln", name=f"wln_{nc.next_id()}")
    nc.scalar.activation(ln, ssq, AF.Ln, bias=0.0, scale=1.0)
    n = work.tile([p, 1], F32, tag="wn", name=f"wn_{nc.next_id()}")
    nc.scalar.activation(n, ln, AF.Exp, bias=0.0, scale=0.5)
    ne = work.tile([p, 1], F32, tag="wne", name=f"wne_{nc.next_id()}")
    nc.vector.tensor_scalar_add(ne, n, W_EPS)
    s = work.tile([p, 1], F32, tag="ws", name=f"ws_{nc.next_id()}")
    nc.vector.reciprocal(s, ne)
    return s


def build_program(split=True):
    nc = bass.Bass()
    x_d = nc.declare_dram_parameter("x", [C, HW], F32, isOutput=False)
    wqn_d = nc.declare_dram_parameter("wqn", [64, C], F32, isOutput=False)
    wkn_d = nc.declare_dram_parameter("wkn", [64, C], F32, isOutput=False)
    wqT_d = nc.declare_dram_parameter("wqT", [C, 64], F32, isOutput=False)
    wkT_d = nc.declare_dram_parameter("wkT", [C, 64], F32, isOutput=False)
    wvT_d = nc.declare_dram_parameter("wvT", [C, 64], F32, isOutput=False)
    won_d = nc.declare_dram_parameter("won", [C, C], F32, isOutput=False)
    woT4_d = nc.declare_dram_parameter("woT4", [128, C], F32, isOutput=False)
    y_d = nc.declare_dram_parameter("y", [C, HW], F32, isOutput=True)
    bq0_d = nc.dram_tensor("bq0", [32, 128], F32)
    bq1_d = nc.dram_tensor("bq1", [32, 128], F32)
    bqs = [bq0_d, bq1_d]

    with tile.TileContext(nc) as tc:
        with tc.tile_pool(name="singles", bufs=1) as sg, \
             tc.tile_pool(name="work", bufs=2) as work, \
             tc.tile_pool(name="scratch", bufs=2) as scr, \
             tc.tile_pool(name="epool", bufs=4) as ep, \
             tc.tile_pool(name="opool", bufs=4) as op, \
             tc.tile_pool(name="scps", bufs=1, space="PSUM") as scps, \
             tc.tile_pool(name="accps", bufs=4, space="PSUM") as accps, \
             tc.tile_pool(name="finps", bufs=2, space="PSUM") as finps:

            # ---------------- P0: loads ----------------
            x_sb = sg.tile([128, 2, HW], F32)
            nc.sync.dma_start(out=x_sb, in_=x_d[:].rearrange("(t p) f -> p t f", p=128))
            wqT_sb = sg.tile([128, 2, 64], F32)
            nc.sync.dma_start(out=wqT_sb, in_=wqT_d[:].rearrange("(t p) m -> p t m", p=128))
            wkT_sb = sg.tile([128, 2, 64], F32)
            nc.sync.dma_start(out=wkT_sb, in_=wkT_d[:].rearrange("(t p) m -> p t m", p=128))
            wvT_sb = sg.tile([128, 2, 64], F32)
            nc.sync.dma_start(out=wvT_sb, in_=wvT_d[:].rearrange("(t p) m -> p t m", p=128))
            wqn_sb = sg.tile([64, C], F32)
            nc.sync.dma_start(out=wqn_sb, in_=wqn_d[:])
            wkn_sb = sg.tile([64, C], F32)
            nc.sync.dma_start(out=wkn_sb, in_=wkn_d[:])
            won_sb = sg.tile([128, 2, C], F32)
            nc.sync.dma_start(out=won_sb, in_=won_d[:].rearrange("(t p) m -> p t m", p=128))
            woT4_sb = sg.tile([128, C], F32)
            nc.sync.dma_start(out=woT4_sb, in_=woT4_d[:])
            ones = sg.tile([128, 128], F32)
            nc.vector.memset(ones, 1.0)
            eps_col = sg.tile([128, 1], F32)
            nc.vector.memset(eps_col, EPS)
            lniq_col = sg.tile([128, 1], F32)
            nc.vector.memset(lniq_col, LOG_ISQ_HD)

            qT = sg.tile([64, HW], F32)
            kT = sg.tile([64, HW], F32)
            va = sg.tile([128, KT * 66], F32)   # per kt: v_h0(32)|sinv_h0|v_h1(32)|sinv_h1
            vbias = sg.tile([128, 64], F32)     # ln(sv), col = kt*2 + h
            skcol = sg.tile([128, 64], F32)     # exp scale, col = kt*2 + h
            sqrow = sg.tile([1, 2 * HW], F32)   # q norm scales row form per head

            # ---------------- P1: weight-norm scales ----------------
            sqq = _weight_scale_rows(nc, work, wqn_sb[:, :], 64)
            sqk = _weight_scale_rows(nc, work, wkn_sb[:, :], 64)
            swo = sg.tile([128, 2], F32)
            wosq = work.tile([128, 2, C], F32, tag="wosq")
            nc.vector.tensor_mul(wosq, won_sb, won_sb)
            wossq = work.tile([128, 2], F32, tag="wossq")
            nc.vector.tensor_reduce(wossq, wosq, axis=mybir.AxisListType.X, op=ALU.add)
            woln = work.tile([128, 2], F32, tag="woln")
            nc.scalar.activation(woln, wossq, AF.Ln, bias=0.0, scale=1.0)
            won_n = work.tile([128, 2], F32, tag="won_n")
            nc.scalar.activation(won_n, woln, AF.Exp, bias=0.0, scale=0.5)
            won_ne = work.tile([128, 2], F32, tag="won_ne")
            nc.vector.tensor_scalar_add(won_ne, won_n, W_EPS)
            swo_inv = work.tile([128, 2], F32, tag="swo_inv")
            nc.vector.reciprocal(swo_inv, won_ne)
            nc.vector.tensor_scalar_mul(swo, swo_inv, float(C_Y))

            # wv column scales: s_v[col] = 1/(||w_v[col]|| + 16 eps), fold into wvT
            wvsq = work.tile([128, 2, 64], F32, tag="wvsq")
            nc.vector.tensor_mul(wvsq, wvT_sb, wvT_sb)
            ssqv_ps = finps.tile([1, 64], F32, tag="fin", name="ssqv_ps")
            for t in range(2):
                nc.tensor.matmul(ssqv_ps, ones[:, 0:1], wvsq[:, t, :],
                                 start=(t == 0), stop=(t == 1))
            vln = work.tile([1, 64], F32, tag="vln")
            nc.scalar.activation(vln, ssqv_ps, AF.Ln, bias=0.0, scale=1.0)
            vn = work.tile([1, 64], F32, tag="vn")
            nc.scalar.activation(vn, vln, AF.Exp, bias=0.0, scale=0.5)
            vne = work.tile([1, 64], F32, tag="vne")
            nc.vector.tensor_scalar_add(vne, vn, W_EPS)
            svrow = work.tile([1, 64], F32, tag="svrow")
            nc.vector.reciprocal(svrow, vne)
            svbc_ps = finps.tile([128, 64], F32, tag="fin", name="svbc_ps")
            nc.tensor.matmul(svbc_ps, ones[0:1, 0:128], svrow[0:1, :],
                             start=True, stop=True)
            for t in range(2):
                nc.vector.tensor_mul(wvT_sb[:, t, :], wvT_sb[:, t, :], svbc_ps)

            # ---------------- P2: convs ----------------
            for ch in range(NCH):
                sl = slice(ch * 512, ch * 512 + 512)
                pq = finps.tile([128, 512], F32, tag="fin", name=f"pq{ch}")
                pk = finps.tile([128, 512], F32, tag="fin", name=f"pk{ch}")
                for t in range(2):
                    nc.tensor.matmul(pq[0:64, :], wqT_sb[:, t, :],
                                     x_sb[:, t, sl], start=(t == 0), stop=(t == 1))
                for t in range(2):
                    nc.tensor.matmul(pk[0:64, :], wkT_sb[:, t, :],
                                     x_sb[:, t, sl], start=(t == 0), stop=(t == 1))
                nc.vector.tensor_scalar(qT[:, sl], pq[0:64, :], sqq[:, 0:1],
                                        None, op0=ALU.mult)
                nc.vector.tensor_scalar(kT[:, sl], pk[0:64, :], sqk[:, 0:1],
                                        None, op0=ALU.mult)
            for pt in range(KT):
                pv = finps.tile([128, 64], F32, tag="fin", name=f"pv{pt}")
                for t in range(2):
                    nc.tensor.matmul(pv, x_sb[:, t, pt * 128:pt * 128 + 128],
                                     wvT_sb[:, t, :], start=(t == 0), stop=(t == 1))
                nc.vector.tensor_copy(va[:, pt * 66:pt * 66 + 32], pv[:, 0:32])
                nc.vector.tensor_copy(va[:, pt * 66 + 33:pt * 66 + 65], pv[:, 32:64])

            # residual pre-scale of x (x only needed for the final add now)
            nc.vector.tensor_scalar_mul(x_sb.rearrange("p t f -> p (t f)"),
                                        x_sb.rearrange("p t f -> p (t f)"), float(C_X))

            # ---------------- P3: v pixel-norm stats ----------------
            va4 = va.rearrange("p (kt h e) -> p kt h e", kt=KT, h=2)
            vsq = scr.tile([128, KT, 2, HD], F32, tag="vsq")
            nc.vector.tensor_mul(vsq, va4[:, :, :, 0:HD], va4[:, :, :, 0:HD])
            msum = sg.tile([128, 64], F32)
            nc.vector.tensor_reduce(msum, vsq, axis=mybir.AxisListType.X, op=ALU.add)
            lnv = sg.tile([128, 64], F32)
            nc.scalar.activation(lnv, msum, AF.Ln, bias=eps_col[:, 0:1], scale=1.0 / HD)
            nc.vector.tensor_scalar_mul(vbias, lnv, -0.5)
            lnv3 = lnv.rearrange("p (kt h) -> p kt h", h=2)
            va3 = va.rearrange("p (kt x) -> p kt x", kt=KT)
            for h in range(2):
                nc.scalar.activation(va3[:, :, 32 + 33 * h:33 + 33 * h],
                                     lnv3[:, :, h:h + 1], AF.Exp, bias=0.0, scale=0.5)

            # ---------------- P4: k exp scales (column form) ----------------
            ksq = scr.tile([64, HW], F32, tag="qksq", name="ksq")
            nc.vector.tensor_mul(ksq, kT, kT)
            psk = finps.tile([128, 64], F32, tag="fin", name="psk")
            for kt in range(KT):
                for h in range(2):
                    nc.tensor.matmul(psk[:, kt * 2 + h:kt * 2 + h + 1],
                                     ksq[32 * h:32 * h + 32, kt * 128:kt * 128 + 128],
                                     ones[32 * h:32 * h + 32, 0:1],
                                     start=True, stop=True)
            lnk = work.tile([128, 64], F32, tag="lnk")
            nc.scalar.activation(lnk, psk, AF.Ln, bias=eps_col[:, 0:1], scale=1.0 / HD)
            nc.scalar.activation(skcol, lnk, AF.Exp, bias=lniq_col[:, 0:1], scale=-0.5)

            # ---------------- P5: q pixel-norm multiply ----------------
            qsq = scr.tile([64, HW], F32, tag="qksq", name="qsq")
            nc.vector.tensor_mul(qsq, qT, qT)
            psq = finps.tile([128, 64], F32, tag="fin", name="psq")
            for h in range(2):
                for t in range(KT):
                    nc.tensor.matmul(psq[:, h * 32 + t:h * 32 + t + 1],
                                     qsq[32 * h:32 * h + 32, t * 128:t * 128 + 128],
                                     ones[32 * h:32 * h + 32, 0:1],
                                     start=True, stop=True)
            lnq = work.tile([128, 64], F32, tag="lnq")
            nc.scalar.activation(lnq, psq, AF.Ln, bias=eps_col[:, 0:1], scale=1.0 / HD)
            sqc = work.tile([128, 64], F32, tag="sqc")
            nc.scalar.activation(sqc, lnq, AF.Exp, bias=0.0, scale=-0.5)
            for h in range(2):
                nc.sync.dma_start(out=bqs[h][:].rearrange("t p -> p t"),
                                  in_=sqc[:, h * 32:h * 32 + 32])
                nc.sync.dma_start(out=sqrow[0:1, h * HW:(h + 1) * HW],
                                  in_=bqs[h][:].rearrange("t p -> (t p)"))
            for ch in range(NCH):
                sl = slice(ch * 512, ch * 512 + 512)
                bch0 = finps.tile([128, 512], F32, tag="fin", name=f"bcq0_{ch}")
                bch1 = finps.tile([128, 512], F32, tag="fin", name=f"bcq1_{ch}")
                nc.tensor.matmul(bch0[0:32, :], ones[0:1, 0:32],
                                 sqrow[0:1, ch * 512:ch * 512 + 512],
                                 start=True, stop=True, tile_position=(0, 0))
                nc.tensor.matmul(bch1[32:64, :], ones[0:1, 0:32],
                                 sqrow[0:1, HW + ch * 512:HW + ch * 512 + 512],
                                 start=True, stop=True, tile_position=(0, 32))
                nc.vector.tensor_mul(qT[0:32, sl], qT[0:32, sl], bch0[0:32, :])
                nc.vector.tensor_mul(qT[32:64, sl], qT[32:64, sl], bch1[32:64, :])

            # ---------------- P6: attention ----------------
            for pr in range(NCH // 2):
                qcs = (2 * pr, 2 * pr + 1)
                acc = {}
                for h in range(2):
                    for j in range(2):
                        acc[(h, j)] = accps.tile([128, 512], F32, tag="acc",
                                                 name=f"acc{pr}_{h}{j}")
                for kt in range(KT):
                    st, sp = kt == 0, kt == KT - 1
                    for h in range(2):
                        hs = slice(32 * h, 32 * h + 32)
                        sc = scps.tile([128, 1024], F32, tag="sc", name=f"sc{pr}_{kt}_{h}")
                        for j in range(2):
                            nc.tensor.matmul(sc[:, j * 512:j * 512 + 512],
                                             kT[hs, kt * 128:kt * 128 + 128],
                                             qT[hs, qcs[j] * 512:qcs[j] * 512 + 512],
                                             start=True, stop=True)
                        e = ep.tile([128, 1024], F32, tag="e", name=f"e{pr}_{kt}_{h}")
                        nc.scalar.activation(e, sc, AF.Exp,
                                             bias=vbias[:, kt * 2 + h:kt * 2 + h + 1],
                                             scale=skcol[:, kt * 2 + h:kt * 2 + h + 1])
                        lhs = va[:, kt * 66 + 33 * h:kt * 66 + 33 * h + 33]
                        for j in range(2):
                            if h == 0:
                                nc.tensor.matmul(acc[(h, j)][0:33, :], lhs,
                                                 e[:, j * 512:j * 512 + 512],
                                                 start=st, stop=sp, tile_position=(0, 0))
                            else:
                                nc.tensor.matmul(acc[(h, j)][64:97, :], lhs,
                                                 e[:, j * 512:j * 512 + 512],
                                                 start=st, stop=sp, tile_position=(0, 64))
                # finalize both chunks of the pair
                for j in range(2):
                    qc = qcs[j]
                    a0, a1 = acc[(0, j)], acc[(1, j)]
                    rden = work.tile([128, 512], F32, tag="rden", name=f"rden{pr}_{j}")
                    nc.vector.reciprocal(rden[32:33, :], a0[32:33, :])
                    nc.vector.reciprocal(rden[96:97, :], a1[96:97, :])
                    bc2a = finps.tile([128, 512], F32, tag="fin", name=f"bc2a_{pr}_{j}")
                    bc2b = finps.tile([128, 512], F32, tag="fin", name=f"bc2b_{pr}_{j}")
                    nc.tensor.matmul(bc2a[0:32, :], ones[32:33, 0:32],
                                     rden[32:33, :], start=True, stop=True,
                                     tile_position=(32, 0))
                    nc.tensor.matmul(bc2b[64:96, :], ones[96:97, 0:32],
                                     rden[96:97, :], start=True, stop=True,
                                     tile_position=(96, 64))
                    yfin = op.tile([128, 512], F32, tag="yfin", name=f"yfin{pr}_{j}")
                    nc.vector.tensor_copy(yfin[0:32, :], a0[0:32, :])
                    nc.vector.tensor_copy(yfin[64:96, :], a1[64:96, :])
                    nc.vector.tensor_mul(yfin[0:32, :], yfin[0:32, :], bc2a[0:32, :])
                    nc.vector.tensor_mul(yfin[64:96, :], yfin[64:96, :], bc2b[64:96, :])
                    # out conv + residual + store
                    for mt in range(2):
                        opsa = finps.tile([128, 512], F32, tag="fin", name=f"opsa{pr}_{j}{mt}")
                        opsb = finps.tile([128, 512], F32, tag="fin", name=f"opsb{pr}_{j}{mt}")
                        nc.tensor.matmul(opsa, woT4_sb[0:32, mt * 128:mt * 128 + 128],
                                         yfin[0:32, :], start=True, stop=True)
                        nc.tensor.matmul(opsb, woT4_sb[64:96, mt * 128:mt * 128 + 128],
                                         yfin[64:96, :], start=True, stop=True)
                        osb = op.tile([128, 512], F32, tag="osb", name=f"osb{pr}_{j}{mt}")
                        nc.vector.scalar_tensor_tensor(
                            osb, opsa, swo[:, mt:mt + 1],
                            x_sb[:, mt, qc * 512:qc * 512 + 512], ALU.mult, ALU.add)
                        nc.vector.scalar_tensor_tensor(
                            osb, opsb, swo[:, mt:mt + 1], osb, ALU.mult, ALU.add)
                        nc.sync.dma_start(
                            out=y_d[:].rearrange("(t p) f -> p t f", p=128)[:, mt:mt + 1, qc * 512:qc * 512 + 512],
                            in_=osb)

    if split:
        _split_waits(nc)
    return nc


_PROG = None
last_results = None


def kernel(x, w_qkv, w_out, num_heads):
    global _PROG
    x = np.asarray(x, dtype=np.float32)
    W = np.asarray(w_qkv, dtype=np.float32)[:, :, 0, 0]
    WO = np.asarray(w_out, dtype=np.float32)[:, :, 0, 0]
    b_, c_, hh, ww = x.shape
    assert (b_, c_, hh * ww) == (2, C, HW)

    if _PROG is None:
        _PROG = build_program()
    nc = _PROG

    in_maps = []
    for core in range(8):
        b = core // 4
        h0 = 2 * (core % 4)
        h1 = h0 + 1
        rq = np.concatenate([W[h0 * HD:(h0 + 1) * HD], W[h1 * HD:(h1 + 1) * HD]], 0)
        rk = np.concatenate([W[C + h0 * HD:C + (h0 + 1) * HD],
                             W[C + h1 * HD:C + (h1 + 1) * HD]], 0)
        rv = np.concatenate([W[2 * C + h0 * HD:2 * C + (h0 + 1) * HD],
                             W[2 * C + h1 * HD:2 * C + (h1 + 1) * HD]], 0)
        woT4 = np.zeros((128, C), np.float32)
        woT4[0:32] = WO[:, h0 * HD:(h0 + 1) * HD].T
        woT4[64:96] = WO[:, h1 * HD:(h1 + 1) * HD].T
        in_maps.append({
            "x": np.ascontiguousarray(x[b].reshape(C, HW)),
            "wqn": np.ascontiguousarray(rq),
            "wkn": np.ascontiguousarray(rk),
            "wqT": np.ascontiguousarray(rq.T),
            "wkT": np.ascontiguousarray(rk.T),
            "wvT": np.ascontiguousarray(rv.T),
            "won": np.ascontiguousarray(WO),
            "woT4": woT4,
        })

    res = run_bass_kernel_spmd(nc, in_maps, list(range(8)))
    global last_results
    last_results = res
    outs = [r["y"] for r in res.results]
    full = np.zeros((2, C, HW), np.float32)
    for core in range(8):
        full[core // 4] += outs[core]
    return full.reshape(b_, c_, hh, ww)

